# revision 1
# baseline (speedup 1.0000x reference)
"""AFM forward on 8 TRN2 NeuronCores: dma_gather (256B quarter-row blocks) + on-chip select.

Math (softmax over the reference's size-1 axis is identically 1, so the
attention branch is dead): out[b] = sigmoid(0.5*w*(||S_b||^2 - Q_b) + bias) with
S_b = sum_f e_{b,f}, Q_b = sum_f ||e_{b,f}||^2.

Gather: per field f one InstDMAGatherAnt with int16 quarter-row indices
qid = id>>2 into table viewed as [25000, 64] 256B rows; each lookup lands
the 4-row block containing its embedding row.  Select on-chip with a 0/1
mask M4[(f,t),r] = (id & 3 == r).

Layouts: batch b = t*128 + p (t in [0,4)).  G/GM [128, f*256 + t*64 + r*16+k].
Host transposes the [128,4] output back to [512,1].
"""

import numpy as np

import concourse.bacc as bacc
import concourse.bass as bass
import concourse.mybir as mybir
from concourse import library_config
from concourse.bass_utils import run_bass_kernel_spmd

N_CORES = 8
B = 4096
NF = 26
EMB = 16
VOCAB = 100000
P = 128
B_SHARD = B // N_CORES      # 512
TT = B_SHARD // P           # 4 slots, b = t*128 + p
ELEM = 64                   # 64 f32 = 256B gathered per lookup
GW = NF * TT * ELEM         # 6656 floats per partition in G
IDXW = NF * 32              # 832 int16 idx cols (26 fields x 512/16)

# hdr1 layout (int32 cols): first 13 fields' idx source + small blocks
AH = 13 * 32                # 416 idx-source cols per half
B0, B1 = AH, AH + NF * TT   # sel ids, col f*4+t
P40, P41 = B1, B1 + 4       # [0,1,2,3]
WB0 = P41                   # w bits, b bits
HDRW = WB0 + 2              # 526
HDR2W = AH                  # hdr2: last 13 fields' idx source

F32 = mybir.dt.float32
I32 = mybir.dt.int32
I16 = mybir.dt.int16
AF = mybir.ActivationFunctionType
NQ = 4                      # SWDGE queues


def build_nc(n_queues: int = NQ) -> bass.Bass:
    nc = bacc.Bacc("TRN2", num_swdge_queues=n_queues)

    hdr_ext = nc.declare_dram_parameter("hdr", [P, HDRW], I32, isOutput=False)
    hdr2_ext = nc.declare_dram_parameter("hdr2", [P, HDR2W], I32, isOutput=False)
    tab_ext = nc.declare_dram_parameter("embed_tables", [NF * VOCAB, EMB], F32, isOutput=False)
    out_ext = nc.declare_dram_parameter("out", [P, TT], F32, isOutput=True)

    from contextlib import ExitStack

    with ExitStack() as ctx:
        hdr = ctx.enter_context(nc.sbuf_tensor([P, HDRW], I32))
        hdr2 = ctx.enter_context(nc.sbuf_tensor([P, HDR2W], I32))
        qidx = ctx.enter_context(nc.sbuf_tensor([P, IDXW], I16))
        qidx32 = ctx.enter_context(nc.sbuf_tensor([P, IDXW], I32))
        sel = ctx.enter_context(nc.sbuf_tensor([P, NF * TT], I32))
        m4 = ctx.enter_context(nc.sbuf_tensor([P, NF * TT * 4], F32))
        g = ctx.enter_context(nc.sbuf_tensor([P, GW], F32))
        gm = ctx.enter_context(nc.sbuf_tensor([P, GW], F32))
        sqs = ctx.enter_context(nc.sbuf_tensor([P, GW], F32))
        s = ctx.enter_context(nc.sbuf_tensor([P, TT * EMB], F32))
        sh = ctx.enter_context(nc.sbuf_tensor([P, TT * 2 * EMB], F32))
        qvh = ctx.enter_context(nc.sbuf_tensor([P, TT * 2], F32))
        s2 = ctx.enter_context(nc.sbuf_tensor([P, TT * EMB], F32))
        ss = ctx.enter_context(nc.sbuf_tensor([P, TT], F32))
        qv = ctx.enter_context(nc.sbuf_tensor([P, TT], F32))
        x = ctx.enter_context(nc.sbuf_tensor([P, TT], F32))
        y = ctx.enter_context(nc.sbuf_tensor([P, TT], F32))
        wh = ctx.enter_context(nc.sbuf_tensor([P, 1], F32))
        d_sem = ctx.enter_context(nc.semaphore("d_sem"))
        vq_sem = ctx.enter_context(nc.semaphore("vq_sem"))
        vq2_sem = ctx.enter_context(nc.semaphore("vq2_sem"))
        d2_sem = ctx.enter_context(nc.semaphore("d2_sem"))
        v_sem = ctx.enter_context(nc.semaphore("v_sem"))
        aq_sem = ctx.enter_context(nc.semaphore("aq_sem"))
        ay_sem = ctx.enter_context(nc.semaphore("ay_sem"))
        # per (queue, half) gather-completion sems, waited at final value only
        gqh = [
            [ctx.enter_context(nc.semaphore(f"gq{q}h{h}")) for h in range(2)]
            for q in range(NQ)
        ]
        # per (t, half) GM-ready sems
        ghsem = [
            [ctx.enter_context(nc.semaphore(f"gh{t}{h}")) for h in range(2)]
            for t in range(TT)
        ]
        block = ctx.enter_context(nc.Block())
        H0 = 13  # fields in group 0
        # work items (field, t0, nt, queue): fields 24/25 split into halves
        # so every queue carries 6.5 gather-slots
        WORK = [(f, 0, TT, f % NQ) for f in range(24)]
        WORK += [(24, 0, 2, 2), (24, 2, 2, 3), (25, 0, 2, 0), (25, 2, 2, 1)]
        gqh_count = [[0] * 2 for _ in range(NQ)]
        for f, t0, nt, qn in WORK:
            gqh_count[qn][0 if f < H0 else 1] += 1
        w_ap = hdr[:, WB0 : WB0 + 1].bitcast(F32)
        b_ap = hdr[:, WB0 + 1 : WB0 + 2].bitcast(F32)

        @block.sync
        def _(sync):
            sync.dma_start(out=hdr[:], in_=hdr_ext[:]).then_inc(d_sem, 16)
            sync.dma_start(out=hdr2[:], in_=hdr2_ext[:]).then_inc(d2_sem, 16)
            sync.wait_ge(ay_sem, 1)
            sync.dma_start(out=out_ext[:], in_=y[:]).then_inc(d_sem, 16)
            sync.wait_ge(d_sem, 32)

        @block.vector
        def _(vector):
            vector.wait_ge(d_sem, 16)
            vector.tensor_scalar(
                out=qidx32[:, :AH],
                in0=hdr[:, 0:AH],
                scalar1=2,
                scalar2=None,
                op0=mybir.AluOpType.logical_shift_right,
            ).then_inc(v_sem, 1)  # v=1
            vector.wait_ge(v_sem, 1)
            vector.tensor_copy(qidx[:, :AH], qidx32[:, :AH]).then_inc(vq_sem, 1)
            vector.tensor_scalar(
                out=sel[:],
                in0=hdr[:, B0:B1],
                scalar1=3,
                scalar2=None,
                op0=mybir.AluOpType.bitwise_and,
            ).then_inc(v_sem, 1)  # v=2
            vector.wait_ge(v_sem, 2)
            vector.tensor_tensor(
                out=m4[:],
                in0=sel[:].rearrange("p (c o) -> p c o", o=1).to_broadcast(
                    [P, NF * TT, 4]
                ),
                in1=hdr[:, P40:P41].rearrange("p (o r) -> p o r", o=1).to_broadcast(
                    [P, NF * TT, 4]
                ),
                op=mybir.AluOpType.is_equal,
            ).then_inc(v_sem, 1)  # v=3
            vector.wait_ge(d2_sem, 16)
            vector.tensor_scalar(
                out=qidx32[:, AH:],
                in0=hdr2[:],
                scalar1=2,
                scalar2=None,
                op0=mybir.AluOpType.logical_shift_right,
            ).then_inc(v_sem, 1)  # v=4
            vector.wait_ge(v_sem, 4)
            vector.tensor_copy(qidx[:, AH:], qidx32[:, AH:]).then_inc(vq2_sem, 1)
            # views: G (f t r k), GM' (t k f r), M4 (f t r)
            g5 = g[:].rearrange("p (f t r k) -> p t f r k", f=NF, t=TT, r=4, k=EMB)
            gm5 = gm[:].rearrange("p (t k f r) -> p t f r k", t=TT, k=EMB, f=NF, r=4)
            m45 = m4[:].rearrange("p (f t r) -> p t f r", f=NF, t=TT, r=4)
            gm_tkc = gm[:].rearrange(
                "p (t k c) -> p t k c", t=TT, k=EMB, c=NF * 4
            )
            vcount = 4
            for h in range(2):
                f0, f1 = (0, H0) if h == 0 else (H0, NF)
                nf_h = f1 - f0
                for q in range(NQ):
                    vector.wait_ge(gqh[q][h], 16 * gqh_count[q][h])
                for t in range(TT):
                    vector.tensor_tensor(
                        out=gm5[:, t : t + 1, f0:f1],
                        in0=g5[:, t : t + 1, f0:f1],
                        in1=m45[:, t : t + 1, f0:f1].to_broadcast(
                            [P, 1, nf_h, 4, EMB]
                        ),
                        op=mybir.AluOpType.mult,
                    ).then_inc(ghsem[t][h], 1)
                for t in range(TT):
                    vector.wait_ge(ghsem[t][h], 1)
                    vector.reduce_sum(
                        sh[:, (t * 2 + h) * EMB : (t * 2 + h + 1) * EMB].rearrange(
                            "p (o k) -> p o k", o=1
                        ),
                        gm_tkc[:, t : t + 1, :, (0 if h == 0 else H0) * 4 : (H0 if h == 0 else NF) * 4],
                        axis=mybir.AxisListType.X,
                    ).then_inc(v_sem, 1)
                    vcount += 1  # v=5..12
            vector.wait_ge(v_sem, 12)
            sh_v = sh[:].rearrange("p (t h k) -> p t h k", t=TT, h=2, k=EMB)
            vector.tensor_tensor(
                out=s[:].rearrange("p (t k) -> p t k", t=TT).rearrange(
                    "p t (o k) -> p t o k", o=1
                ),
                in0=sh_v[:, :, 0:1],
                in1=sh_v[:, :, 1:2],
                op=mybir.AluOpType.add,
            ).then_inc(v_sem, 1)  # v=13
            vector.wait_ge(v_sem, 13)
            vector.tensor_mul(s2[:], s[:], s[:]).then_inc(v_sem, 1)  # v=14
            vector.wait_ge(v_sem, 14)
            vector.reduce_sum(
                ss[:],
                s2[:].rearrange("p (t k) -> p t k", t=TT),
                axis=mybir.AxisListType.X,
            ).then_inc(v_sem, 1)  # v=15
            vector.tensor_scalar_mul(wh[:], w_ap, 0.5).then_inc(v_sem, 1)  # v=16
            vector.wait_ge(aq_sem, TT * 2)
            qvh_v = qvh[:].rearrange("p (t h) -> p t h", t=TT, h=2)
            vector.tensor_tensor(
                out=qv[:].rearrange("p (t o) -> p t o", o=1),
                in0=qvh_v[:, :, 0:1],
                in1=qvh_v[:, :, 1:2],
                op=mybir.AluOpType.add,
            ).then_inc(v_sem, 1)  # v=17
            vector.wait_ge(v_sem, 17)
            vector.tensor_tensor(
                x[:], ss[:], qv[:], op=mybir.AluOpType.subtract
            ).then_inc(v_sem, 1)  # v=18 (final)

        @block.gpsimd
        def _(gpsimd):
            gpsimd.load_library(library_config.mlp)
            gpsimd.wait_ge(vq_sem, 1)
            for f, t0, nt, qn in WORK:
                if f == 13 and t0 == 0:
                    gpsimd.wait_ge(vq2_sem, 1)
                slab = tab_ext[f * VOCAB : (f + 1) * VOCAB, :].rearrange(
                    "(a b) k -> a (b k)", b=4
                )  # [25000, 64] 256B rows
                gpsimd.dma_gather(
                    out_ap=g[:, f * TT * ELEM : (f + 1) * TT * ELEM].rearrange(
                        "p (t e) -> p t e", e=ELEM
                    )[:, t0 : t0 + nt, :],
                    in_ap=slab,
                    idxs_ap=qidx[:, f * 32 + t0 * 8 : f * 32 + (t0 + nt) * 8],
                    num_idxs=nt * P,
                    num_idxs_reg=nt * P,
                    elem_size=ELEM,
                    queue_num=qn,
                    single_packet=False,
                ).then_inc(gqh[qn][0 if f < H0 else 1], 16)

        @block.scalar
        def _(scalar):
            scalar.wait_ge(d_sem, 16)
            gm_a = gm[:].rearrange(
                "p (t k c) -> p t k c", t=TT, k=EMB, c=NF * 4
            )
            sqs_a = sqs[:].rearrange(
                "p (t k c) -> p t k c", t=TT, k=EMB, c=NF * 4
            )
            for h in range(2):
                c0, c1 = (0, H0 * 4) if h == 0 else (H0 * 4, NF * 4)
                for t in range(TT):
                    scalar.wait_ge(ghsem[t][h], 1)
                    scalar.activation(
                        sqs_a[:, t : t + 1, :, c0:c1],
                        gm_a[:, t : t + 1, :, c0:c1],
                        AF.Square,
                        accum_out=qvh[:, t * 2 + h : t * 2 + h + 1],
                    ).then_inc(aq_sem, 1)
            scalar.wait_ge(v_sem, 18)
            scalar.activation(
                y[:], x[:], AF.Sigmoid, bias=b_ap, scale=wh[:]
            ).then_inc(ay_sem, 1)

    nc.compile()
    return nc


_NC_CACHE = None


def _get_nc() -> bass.Bass:
    global _NC_CACHE
    if _NC_CACHE is None:
        _NC_CACHE = build_nc()
    return _NC_CACHE


def make_hdr(ids_shard: np.ndarray, w: np.float32, bb: np.float32):
    """ids_shard [512, 26] int32 -> (hdr1 [128, 526], hdr2 [128, 416]) int32."""
    a = np.zeros((P, NF * 32), dtype=np.int32)
    j = np.arange(B_SHARD)
    for f in range(NF):
        blk = np.zeros((16, 32), dtype=np.int32)
        blk[j % 16, j // 16] = ids_shard[:, f]
        a[:, f * 32 : (f + 1) * 32] = np.tile(blk, (8, 1))
    hdr = np.zeros((P, HDRW), dtype=np.int32)
    hdr[:, 0:AH] = a[:, :AH]
    sel = ids_shard.reshape(TT, P, NF).transpose(1, 2, 0)  # [p, f, t]
    hdr[:, B0:B1] = sel.reshape(P, NF * TT)
    hdr[:, P40:P41] = np.arange(4, dtype=np.int32)[None, :]
    hdr[:, WB0 : WB0 + 2] = np.array([[w, bb]], dtype=np.float32).view(np.int32)
    return hdr, np.ascontiguousarray(a[:, AH:])


def make_in_maps(inputs: dict) -> list[dict]:
    ids = np.ascontiguousarray(np.asarray(inputs["sparse_ids"], dtype=np.int32))
    tab = np.ascontiguousarray(
        np.asarray(inputs["embed_tables"], dtype=np.float32)
    ).reshape(NF * VOCAB, EMB)
    w = np.float32(np.asarray(inputs["out_kernel"]).reshape(()))
    bb = np.float32(np.asarray(inputs["out_bias"]).reshape(()))
    maps = []
    for c in range(N_CORES):
        h1, h2 = make_hdr(ids[c * B_SHARD : (c + 1) * B_SHARD], w, bb)
        maps.append({"hdr": h1, "hdr2": h2, "embed_tables": tab})
    return maps


def run(inputs: dict, **spmd_kwargs):
    nc = _get_nc()
    in_maps = make_in_maps(inputs)
    res = run_bass_kernel_spmd(nc, in_maps, core_ids=list(range(N_CORES)), **spmd_kwargs)
    outs = []
    for i in range(N_CORES):
        yv = np.asarray(res.results[i]["out"], dtype=np.float32).reshape(P, TT)
        outs.append(yv.T.reshape(B_SHARD, 1))  # b = t*128 + p
    return np.concatenate(outs, axis=0), res


def kernel(**inputs) -> np.ndarray:
    out, _ = run(inputs)
    return out



# revision 4
# speedup vs baseline: 1.0385x; 1.0385x over previous
"""AFM forward: fp16 tables, 14 pair-gathers, cheap on-chip select.

Math (softmax over size-1 axis == 1): out[b] = sigmoid(0.5*w*(||S_b||^2 - Q_b) + b0)
with S_b = sum_f e_{b,f}, Q_b = sum_f ||e_{b,f}||^2.

Gather: tables converted to fp16 on host. For pair g (fields 2g, 2g+1) one
InstDMAGatherAnt over slab [25000, 128] fp16 (256B rows = 8 embedding rows).
int16 idx = f_local*12500 + (id>>3) <= 24999. Pair 0 runs as 2x512-idx
gathers (the one-time SWDGE cold-start tax scales with the first
instruction's index count); pairs 1-12 as 1024-idx gathers. Shared
num_idxs registers avoid per-gather MOVEs.

Select: m8[f,t,r] = (id&7 == r) on vector (small), broadcast k-packed to m8k
on the SCALAR engine (activation Copy), masked mult G*m8k in fp16 2x DVE
mode, 3-level fp16 add tree over r (exact: 7 of 8 summands zero), then
per-chunk f-trees accumulate S partials (no big strided reduce at the end).
Q via e^2 + one XY-reduce per chunk.

Layouts (per-partition free offsets): batch b = t*128 + p.
G/m8k/GM [P, (f t r k)]  f:512, t:128, r:16, k:1 (fp16).
"""

import numpy as np

import concourse.bacc as bacc
import concourse.bass as bass
import concourse.mybir as mybir
from concourse import library_config
from concourse.bass_utils import run_bass_kernel_spmd

N_CORES = 8
B = 4096
NF = 26
EMB = 16
VOCAB = 100000
P = 128
B_SHARD = B // N_CORES      # 512
TT = B_SHARD // P           # 4 slots, b = t*128 + p
NPAIR = NF // 2             # 13 gathers
RD = 8                      # rows per 256B fp16 block
BLK = RD * EMB              # 128 fp16 elems per gathered block
GW = NF * TT * BLK          # 13312 fp16 per partition

# hdr layout (int32 cols)
QW = NPAIR * 64 // 2        # 416 i32 cols of packed int16 idxs
SEL0 = QW                   # 104 cols: sel = id&7, col SEL0 + f*4 + t
IOTA0 = SEL0 + NF * TT      # 8 cols: 0..7
WB0 = IOTA0 + 8             # 2 cols: [0.5*w, bias] f32 bits
HDRW = WB0 + 2              # 530

CHUNK_PAIRS = [4, 4, 4, 1]          # pairs per chunk
CHUNK_F = [(0, 8), (8, 16), (16, 24), (24, 26)]
CHUNK_OF = [0] * 4 + [1] * 4 + [2] * 4 + [3] * 1
NCH = 4

F32 = mybir.dt.float32
F16 = mybir.dt.float16
I32 = mybir.dt.int32
I16 = mybir.dt.int16
AF = mybir.ActivationFunctionType
NQ = 4                      # SWDGE queues (ucode max)


def build_nc(n_queues: int = NQ) -> bass.Bass:
    nc = bacc.Bacc("TRN2", num_swdge_queues=n_queues)

    hdr_ext = nc.declare_dram_parameter("hdr", [P, HDRW], I32, isOutput=False)
    tab_ext = nc.declare_dram_parameter("embed_tables", [NF * VOCAB, EMB], F16, isOutput=False)
    out_ext = nc.declare_dram_parameter("out", [P, TT], F32, isOutput=True)

    from contextlib import ExitStack

    with ExitStack() as ctx:
        hdr = ctx.enter_context(nc.sbuf_tensor([P, HDRW], I32))
        m8 = ctx.enter_context(nc.sbuf_tensor([P, NF * TT * RD], F32))
        m8k = ctx.enter_context(nc.sbuf_tensor([P, GW], F16))
        g = ctx.enter_context(nc.sbuf_tensor([P, GW], F16))
        gm = ctx.enter_context(nc.sbuf_tensor([P, GW], F16))
        t1 = ctx.enter_context(nc.sbuf_tensor([P, GW // 2], F16))
        t2 = ctx.enter_context(nc.sbuf_tensor([P, GW // 4], F16))
        e = ctx.enter_context(nc.sbuf_tensor([P, NF * TT * EMB], F16))
        e2 = ctx.enter_context(nc.sbuf_tensor([P, NF * TT * EMB], F16))
        sc1 = ctx.enter_context(nc.sbuf_tensor([P, 4 * TT * EMB], F16))
        sc2 = ctx.enter_context(nc.sbuf_tensor([P, 2 * TT * EMB], F16))
        sp = ctx.enter_context(nc.sbuf_tensor([P, NCH * TT * EMB], F16))
        sa = ctx.enter_context(nc.sbuf_tensor([P, 2 * TT * EMB], F16))
        s = ctx.enter_context(nc.sbuf_tensor([P, TT * EMB], F32))
        s2 = ctx.enter_context(nc.sbuf_tensor([P, TT * EMB], F32))
        qcell = ctx.enter_context(nc.sbuf_tensor([P, TT * NCH], F32))
        qv = ctx.enter_context(nc.sbuf_tensor([P, TT], F32))
        ss = ctx.enter_context(nc.sbuf_tensor([P, TT], F32))
        x = ctx.enter_context(nc.sbuf_tensor([P, TT], F32))
        y = ctx.enter_context(nc.sbuf_tensor([P, TT], F32))
        d_sem = ctx.enter_context(nc.semaphore("d_sem"))
        vseq = ctx.enter_context(nc.semaphore("vseq"))
        vm8 = ctx.enter_context(nc.semaphore("vm8"))
        sm8k = ctx.enter_context(nc.semaphore("sm8k"))
        vx = ctx.enter_context(nc.semaphore("vx"))
        ay = ctx.enter_context(nc.semaphore("ay"))
        psem = [ctx.enter_context(nc.semaphore(f"ps{gp}")) for gp in range(NPAIR)]
        block = ctx.enter_context(nc.Block())

        wh_ap = hdr[:, WB0 : WB0 + 1].bitcast(F32)
        b_ap = hdr[:, WB0 + 1 : WB0 + 2].bitcast(F32)

        @block.sync
        def _(sync):
            sync.dma_start(out=hdr[:], in_=hdr_ext[:]).then_inc(d_sem, 16)
            sync.wait_ge(ay, 1)
            sync.dma_start(out=out_ext[:], in_=y[:]).then_inc(d_sem, 16)
            sync.wait_ge(d_sem, 32)

        @block.vector
        def _(vector):
            vcnt = [0]

            def step(inst):
                inst.then_inc(vseq, 1)
                vcnt[0] += 1
                return vcnt[0]

            vector.wait_ge(d_sem, 16)
            # m8[f,t,r] = (sel[f,t] == r)
            vector.tensor_tensor(
                out=m8[:].rearrange("p (f t r) -> p f t r", f=NF, t=TT, r=RD),
                in0=hdr[:, SEL0:IOTA0]
                .rearrange("p (f t o) -> p f t o", f=NF, t=TT, o=1)
                .to_broadcast([P, NF, TT, RD]),
                in1=hdr[:, IOTA0 : IOTA0 + 8]
                .rearrange("p (a b r) -> p a b r", a=1, b=1)
                .to_broadcast([P, NF, TT, RD]),
                op=mybir.AluOpType.is_equal,
            ).then_inc(vm8, 1)

            gm5 = gm[:].rearrange("p (f t r k) -> p f t r k", f=NF, t=TT, r=RD, k=EMB)
            t15 = t1[:].rearrange("p (f t r k) -> p f t r k", f=NF, t=TT, r=4, k=EMB)
            t25 = t2[:].rearrange("p (f t r k) -> p f t r k", f=NF, t=TT, r=2, k=EMB)
            e5 = e[:].rearrange("p (f t o k) -> p f t o k", f=NF, t=TT, o=1, k=EMB)
            ev = e[:].rearrange("p (f t k) -> p f t k", f=NF, t=TT, k=EMB)
            e24 = e2[:].rearrange("p (f t k) -> p f t k", f=NF, t=TT, k=EMB)
            sc1v = sc1[:].rearrange("p (f t k) -> p f t k", f=4, t=TT, k=EMB)
            sc2v = sc2[:].rearrange("p (f t k) -> p f t k", f=2, t=TT, k=EMB)
            spv = sp[:].rearrange("p (c t k) -> p c t k", c=NCH, t=TT, k=EMB)
            sav = sa[:].rearrange("p (c t k) -> p c t k", c=2, t=TT, k=EMB)

            for c, (f0, f1) in enumerate(CHUNK_F):
                nf = f1 - f0
                vector.wait_ge(sm8k, c + 1)
                for gp in range(NPAIR):
                    if CHUNK_OF[gp] == c:
                        vector.wait_ge(psem[gp], 32 if gp == 0 else 16)
                n = step(vector.tensor_tensor(
                    out=gm[:, f0 * TT * BLK : f1 * TT * BLK],
                    in0=g[:, f0 * TT * BLK : f1 * TT * BLK],
                    in1=m8k[:, f0 * TT * BLK : f1 * TT * BLK],
                    op=mybir.AluOpType.mult,
                ))
                vector.wait_ge(vseq, n)
                n = step(vector.tensor_tensor(
                    out=t15[:, f0:f1],
                    in0=gm5[:, f0:f1, :, 0:4],
                    in1=gm5[:, f0:f1, :, 4:8],
                    op=mybir.AluOpType.add,
                ))
                vector.wait_ge(vseq, n)
                n = step(vector.tensor_tensor(
                    out=t25[:, f0:f1],
                    in0=t15[:, f0:f1, :, 0:2],
                    in1=t15[:, f0:f1, :, 2:4],
                    op=mybir.AluOpType.add,
                ))
                vector.wait_ge(vseq, n)
                n = step(vector.tensor_tensor(
                    out=e5[:, f0:f1],
                    in0=t25[:, f0:f1, :, 0:1],
                    in1=t25[:, f0:f1, :, 1:2],
                    op=mybir.AluOpType.add,
                ))
                vector.wait_ge(vseq, n)
                # Q partials: qcell[t,c] = sum over (f in chunk, k) of e^2
                n = step(vector.tensor_mul(
                    e2[:, f0 * TT * EMB : f1 * TT * EMB],
                    e[:, f0 * TT * EMB : f1 * TT * EMB],
                    e[:, f0 * TT * EMB : f1 * TT * EMB],
                ))
                # S partial for this chunk via f-tree -> sp[c]
                if nf == 8:
                    n2 = step(vector.tensor_add(
                        sc1v[:, :], ev[:, f0 : f0 + 4], ev[:, f0 + 4 : f0 + 8]
                    ))
                    vector.wait_ge(vseq, n2)
                    n2 = step(vector.tensor_add(
                        sc2v[:, :], sc1v[:, 0:2], sc1v[:, 2:4]
                    ))
                    vector.wait_ge(vseq, n2)
                    n2 = step(vector.tensor_add(
                        spv[:, c : c + 1], sc2v[:, 0:1], sc2v[:, 1:2]
                    ))
                else:  # nf == 2
                    n2 = step(vector.tensor_add(
                        spv[:, c : c + 1], ev[:, f0 : f0 + 1], ev[:, f0 + 1 : f0 + 2]
                    ))
                vector.wait_ge(vseq, n)  # e2 ready
                step(vector.reduce_sum(
                    qcell[:].rearrange("p (t c) -> p t c", t=TT, c=NCH)[:, :, c : c + 1],
                    e24[:, f0:f1].rearrange("p f t k -> p t f k"),
                    axis=mybir.AxisListType.XY,
                ))
                if c == 1:
                    # fold chunk-0/1 S partials now, off the critical tail
                    vector.wait_ge(vseq, vcnt[0])
                    step(vector.tensor_add(
                        sav[:, 0:1], spv[:, 0:1], spv[:, 1:2]
                    ))

            # S = sum of chunk partials (sa[0] was folded after chunk 1)
            vector.wait_ge(vseq, vcnt[0])
            n = step(vector.tensor_add(sav[:, 1:2], spv[:, 2:3], spv[:, 3:4]))
            vector.wait_ge(vseq, n)
            n = step(vector.tensor_add(
                s[:].rearrange("p (o t k) -> p o t k", o=1, t=TT, k=EMB),
                sav[:, 0:1],
                sav[:, 1:2],
            ))
            vector.wait_ge(vseq, n)
            n = step(vector.tensor_mul(s2[:], s[:], s[:]))
            vector.wait_ge(vseq, n)
            n = step(vector.reduce_sum(
                ss[:],
                s2[:].rearrange("p (t k) -> p t k", t=TT, k=EMB),
                axis=mybir.AxisListType.X,
            ))
            # qv[t] = sum_c qcell[t,c]
            n = step(vector.reduce_sum(
                qv[:],
                qcell[:].rearrange("p (t c) -> p t c", t=TT, c=NCH),
                axis=mybir.AxisListType.X,
            ))
            vector.wait_ge(vseq, n)
            vector.tensor_tensor(
                x[:], ss[:], qv[:], op=mybir.AluOpType.subtract
            ).then_inc(vx, 1)

        @block.scalar
        def _(scalar):
            scalar.wait_ge(d_sem, 16)
            scalar.wait_ge(vm8, 1)
            m84 = m8[:].rearrange("p (f t r o) -> p f t r o", f=NF, t=TT, r=RD, o=1)
            m8k5 = m8k[:].rearrange(
                "p (f t r k) -> p f t r k", f=NF, t=TT, r=RD, k=EMB
            )
            for c, (f0, f1) in enumerate(CHUNK_F):
                scalar.activation(
                    m8k5[:, f0:f1],
                    m84[:, f0:f1].to_broadcast([P, f1 - f0, TT, RD, EMB]),
                    AF.Copy,
                ).then_inc(sm8k, 1)
            scalar.wait_ge(vx, 1)
            scalar.activation(
                y[:], x[:], AF.Sigmoid, bias=b_ap, scale=wh_ap
            ).then_inc(ay, 1)

        @block.gpsimd
        def _(gpsimd):
            gpsimd.load_library(library_config.mlp)
            gpsimd.wait_ge(d_sem, 16)
            r_half = gpsimd.to_reg(TT * P)
            r_full = gpsimd.to_reg(2 * TT * P)
            qidx = hdr[:, 0:QW].bitcast(I16)  # [P, 832]
            # pair 0 as 2 x 512-idx gathers: the one-time cold-start tax of the
            # gather ucode scales with the first instruction's index count
            slab0 = tab_ext[0 : 2 * VOCAB, :].rearrange("(a b) k -> a (b k)", b=RD)
            for h in range(2):
                gpsimd.dma_gather(
                    out_ap=g[:, h * TT * BLK : (h + 1) * TT * BLK].rearrange(
                        "p (i e) -> p i e", e=BLK
                    ),
                    in_ap=slab0,
                    idxs_ap=qidx[:, h * 32 : (h + 1) * 32],
                    num_idxs=TT * P,
                    num_idxs_reg=r_half,
                    elem_size=BLK,
                    queue_num=0,
                    single_packet=False,
                ).then_inc(psem[0], 16)
            for gp in range(1, NPAIR):
                slab = tab_ext[2 * gp * VOCAB : (2 * gp + 2) * VOCAB, :].rearrange(
                    "(a b) k -> a (b k)", b=RD
                )  # [25000, 128] fp16 = 256B rows
                gpsimd.dma_gather(
                    out_ap=g[:, gp * 2 * TT * BLK : (gp + 1) * 2 * TT * BLK].rearrange(
                        "p (i e) -> p i e", e=BLK
                    ),
                    in_ap=slab,
                    idxs_ap=qidx[:, gp * 64 : (gp + 1) * 64],
                    num_idxs=2 * TT * P,
                    num_idxs_reg=r_full,
                    elem_size=BLK,
                    queue_num=gp % NQ,
                    single_packet=False,
                ).then_inc(psem[gp], 16)

    nc.compile()
    return nc


_NC_CACHE = None


def _get_nc() -> bass.Bass:
    global _NC_CACHE
    if _NC_CACHE is None:
        _NC_CACHE = build_nc()
    return _NC_CACHE


def make_hdr(ids_shard: np.ndarray, wh: np.float32, bb: np.float32) -> np.ndarray:
    """ids_shard [512, 26] int32 -> hdr [128, 530] int32."""
    qidx = np.zeros((P, NPAIR * 64), dtype=np.int16)
    j = np.arange(2 * B_SHARD)
    for gp in range(NPAIR):
        q0 = (ids_shard[:, 2 * gp] >> 3).astype(np.int16)
        q1 = (12500 + (ids_shard[:, 2 * gp + 1] >> 3)).astype(np.int16)
        cat = np.concatenate([q0, q1])
        blk = np.zeros((16, 64), dtype=np.int16)
        blk[j % 16, j // 16] = cat
        qidx[:, gp * 64 : (gp + 1) * 64] = np.tile(blk, (8, 1))
    hdr = np.zeros((P, HDRW), dtype=np.int32)
    hdr[:, 0:QW] = qidx.view(np.int32)
    sel = (ids_shard & 7).reshape(TT, P, NF).transpose(1, 2, 0)  # [p, f, t]
    hdr[:, SEL0:IOTA0] = sel.reshape(P, NF * TT)
    hdr[:, IOTA0 : IOTA0 + 8] = np.arange(8, dtype=np.int32)[None, :]
    hdr[:, WB0 : WB0 + 2] = np.array([[wh, bb]], dtype=np.float32).view(np.int32)
    return hdr


def make_in_maps(inputs: dict) -> list[dict]:
    ids = np.ascontiguousarray(np.asarray(inputs["sparse_ids"], dtype=np.int32))
    tab = np.ascontiguousarray(
        np.asarray(inputs["embed_tables"]).astype(np.float16)
    ).reshape(NF * VOCAB, EMB)
    w = np.float32(np.asarray(inputs["out_kernel"]).reshape(()))
    bb = np.float32(np.asarray(inputs["out_bias"]).reshape(()))
    wh = np.float32(0.5 * w)
    maps = []
    for c in range(N_CORES):
        h = make_hdr(ids[c * B_SHARD : (c + 1) * B_SHARD], wh, bb)
        maps.append({"hdr": h, "embed_tables": tab})
    return maps


def run(inputs: dict, **spmd_kwargs):
    nc = _get_nc()
    in_maps = make_in_maps(inputs)
    res = run_bass_kernel_spmd(nc, in_maps, core_ids=list(range(N_CORES)), **spmd_kwargs)
    outs = []
    for i in range(N_CORES):
        yv = np.asarray(res.results[i]["out"], dtype=np.float32).reshape(P, TT)
        outs.append(yv.T.reshape(B_SHARD, 1))  # b = t*128 + p
    return np.concatenate(outs, axis=0), res


def kernel(**inputs) -> np.ndarray:
    out, _ = run(inputs)
    return out
